# revision 45
# baseline (speedup 1.0000x reference)
"""Distributed GAT GNN kernel for 8 TRN2 NeuronCores (self-contained).

Algorithm (per core c, SPMD single program, per-core data via inputs):
  - T1[n] = [x@W1 | x@W1As1 | x@W1Ad1]  (bf16 table, replicated compute)
  - Layer-1 message passing for own dst nodes [c*6250, (c+1)*6250):
    dst-sorted edges grouped per PAIR of 128-dst blocks; per pair one
    dma_gather per src window (int16 index range) fetches [h|a_s] rows;
    attention softmax without segment-max (scores tiny); aggregation via
    one-hot matmuls accumulated in PSUM; denominators ride as extra
    matmul columns.
  - T2 rows for own nodes built inside the layer-1 loop (PE transpose of
    h2 + matmul with folded W2/BN) into SBUF tile T2own; AllGather of T2
    runs in two chunks overlapped under the layer-1 tail.  Layer-2
    self-loops are served from T2own instead of the gather.
  - Layer-2 message passing + per-graph pooling via one-hot matmuls,
    two-chunk AllReduce of pooled sums, tiny MLP head. Output [500, 2].
"""
import sys

import numpy as np
from ml_dtypes import bfloat16

for _p in ("/opt/trn_rl_repo",):
    if _p not in sys.path:
        sys.path.append(_p)

import concourse.bass as bass
import concourse.tile as tile
from concourse import bacc, bass_utils, mybir

F32 = mybir.dt.float32
BF16 = mybir.dt.bfloat16
I16 = mybir.dt.int16
AF = mybir.ActivationFunctionType
OP = mybir.AluOpType

N = 50000
F_IN = 128
HID = 64
HEADS = 4
HC = HEADS * HID            # 256
OUT_DIM = 128
N_CLS = 2
NG = 500
SLOPE = 0.2
EPS = 1e-5
NCORES = 8
NLOC = N // NCORES          # 6250
NBLK = (NLOC + 127) // 128  # 49
LAST_VALID = NLOC - (NBLK - 1) * 128  # 106
TCOLS = 384                 # bf16 table row stride (768B)
UCOLS = 264                 # used columns [h(256)|a_s(4)|a_d(4)]
NPAD = 50176                # 98*512
NPAD_LOC = NBLK * 128       # 6272
SPLIT1 = 25088              # layer-1 src window boundary (int16 range)
# T2 AllGather runs in 4 progressively smaller chunks, each issued a few
# blocks after its data is complete, so only a tiny tail is exposed:
#   chunk A  = blocks 0..23  -> table T2A; B1/B2/B3 -> sections of T2BC
NBLK_A = 24
ROWS_A = NBLK_A * 128       # 3072
ROWS_B = NLOC - ROWS_A      # 3178
# B sections (local rows relative to ROWS_A): B1 blocks 24..35, B2 36..43,
# B3 44..48
BSEC = [12 * 128, 8 * 128, NLOC - 44 * 128]   # 1536, 1024, 618
BSEC_OFF = [0, 1536, 2560]                     # local row offset in T2lB
BSEC_BASE = [0, NCORES * 1536, NCORES * 2560]  # row base in T2BC
AG_AFTER_PAIR = {27: 0, 39: 1, 47: 2}  # pair idx -> AllGather (A=0.. fires
                                       # for T2A at 27, B1 at 39, B2 at 47)
AR1_AFTER_PAIR = 30         # issue AllReduce#1 (pool) after this L2 group
POOL_SPLIT_BLK = 25         # pooling accumulation split block
PAIRS = [(b,) for b in range(NBLK)]   # per-block gather groups


def _bf(x):
    return np.ascontiguousarray(np.asarray(x, np.float32).astype(bfloat16))


def _f32(x):
    return np.ascontiguousarray(np.asarray(x, np.float32))


# ---------------------------------------------------------------- host prep
def _layer_lists(s, d, in_A, row_A, row_B):
    """Per core, per block, per part: (table_rows, dst_slots) sorted by slot."""
    lists = [[None] * NBLK for _ in range(NCORES)]
    nA = np.zeros(NBLK, np.int64)
    nB = np.zeros(NBLK, np.int64)
    core_of = d // NLOC
    for c in range(NCORES):
        m = core_of == c
        ss, dd = s[m], d[m] - c * NLOC
        o = np.argsort(dd, kind="stable")
        ss, dd = ss[o], dd[o]
        blk = dd // 128
        bnd = np.searchsorted(blk, np.arange(NBLK + 1))
        for b in range(NBLK):
            sb = ss[bnd[b]:bnd[b + 1]]
            db = dd[bnd[b]:bnd[b + 1]] - b * 128
            mA = in_A(sb)
            lists[c][b] = ((row_A(sb[mA]), db[mA]),
                           (row_B(sb[~mA]), db[~mA]))
            nA[b] = max(nA[b], (len(sb[mA]) + 127) // 128)
            nB[b] = max(nB[b], (len(sb) - len(sb[mA]) + 127) // 128)
    return lists, nA, nB


def _assemble_layer(lists, nA, nB):
    """Flat chunk order per pair: [A(b0), A(b1), B(b0), B(b1)].

    Returns pair metadata + per-core idx16 / dslot / oTt arrays.
    Each call's index list is 0-padded to the max real count over cores
    (num_idxs_reg) and -1-padded to the chunk-aligned size beyond that
    (trailing -1 descriptors are skipped by the Q7 kernel).
    """
    # per (block, part): max real edge count over cores
    mc = np.zeros((NBLK, 2), np.int64)
    for c in range(NCORES):
        for b in range(NBLK):
            for part in (0, 1):
                mc[b, part] = max(mc[b, part], len(lists[c][b][part][0]))

    pair_info = []
    chunk_block = []            # block id per flat chunk
    q = 0
    for pr in PAIRS:
        info = dict(q0=q, blocks=[])
        aoff = {}
        boff = {}
        for b in pr:
            aoff[b] = q
            q += int(nA[b])
            chunk_block += [b] * int(nA[b])
        for b in pr:
            boff[b] = q
            q += int(nB[b])
            chunk_block += [b] * int(nB[b])
        for b in pr:
            info["blocks"].append(dict(
                b=b, a0=aoff[b], na=int(nA[b]), b0=boff[b], nb=int(nB[b])))
        info["G"] = q - info["q0"]
        info["gA"] = sum(int(nA[b]) for b in pr)
        info["gB"] = sum(int(nB[b]) for b in pr)
        # all pad slots gather row 0 (must be written: unwritten slots
        # would inject NaN via 0*NaN in the aggregation matmul)
        info["nidxA"] = info["gA"] * 128
        info["nidxB"] = info["gB"] * 128
        pair_info.append(info)
    CH = q
    L = CH * 128

    idx16_l, dslot_l, oTt_l = [], [], []
    dvals = np.arange(128, dtype=np.float32)[:, None]
    for c in range(NCORES):
        idx = np.zeros(L, np.int64)
        slo = np.full(L, -1.0, np.float32)
        for info in pair_info:
            for part in (0, 1):
                blks = info["blocks"]
                for i, blk in enumerate(blks):
                    off = (blk["a0"] if part == 0 else blk["b0"]) * 128
                    g = blk["na"] if part == 0 else blk["nb"]
                    rows, slots = lists[c][blk["b"]][part]
                    n = len(rows)
                    assert n <= g * 128
                    idx[off:off + n] = rows
                    slo[off:off + n] = slots.astype(np.float32)
        idx16_l.append(np.tile(
            idx.astype(np.int16).reshape(L // 16, 16).T, (8, 1)))
        dslot_l.append(slo.reshape(CH, 128).T.astype(bfloat16))
        oTt_l.append((slo[None, :] == dvals).astype(bfloat16))
    return dict(pairs=pair_info, CH=CH, L=L, chunk_block=chunk_block,
                idx16=idx16_l, dslot=dslot_l, oTt=oTt_l)


def preprocess_graph(edge_index, batch):
    src = np.asarray(edge_index[0], np.int64)
    dst = np.asarray(edge_index[1], np.int64)
    loop = np.arange(N, dtype=np.int64)

    # layer 1: edges + self loops all go through the gather on table T1.
    # T1 is stored partition-major (node n -> physical row
    # (n%128)*392 + (n//512)*4 + (n%512)//128) so the build writes are
    # per-partition sequential streams; the int16 window split then falls
    # exactly at partition 64 (phys row 25088).
    s1 = np.concatenate([src, loop])
    d1 = np.concatenate([dst, loop])

    def phys1(n):
        return (n % 128) * (NPAD // 128) + (n // 512) * 4 + (n % 512) // 128

    l1 = _assemble_layer(*_layer_lists(
        s1, d1, lambda s: (s % 128) < 64,
        lambda s: phys1(s), lambda s: phys1(s) - SPLIT1))

    # layer 2: self loops served from SBUF (T2own); gather on chunked
    # tables T2A (local rows < ROWS_A) / T2BC (B1 then B2 sections).
    def row_B(s):
        c = s // NLOC
        r = s % NLOC - ROWS_A
        out = np.zeros_like(s)
        for off, n, base in zip(BSEC_OFF, BSEC, BSEC_BASE):
            m = (r >= off) & (r < off + n)
            out = np.where(m, base + c * n + (r - off), out)
        return out

    l2 = _assemble_layer(*_layer_lists(
        src, dst,
        lambda s: (s % NLOC) < ROWS_A,
        lambda s: (s // NLOC) * ROWS_A + (s % NLOC),
        row_B))

    batch = np.asarray(batch, np.int64)
    bslot_l = []
    for c in range(NCORES):
        bs = np.full((128, NBLK), -1.0, np.float32)
        loc = batch[c * NLOC:(c + 1) * NLOC]
        for b in range(NBLK):
            seg = loc[b * 128:(b + 1) * 128]
            bs[:len(seg), b] = seg.astype(np.float32)
        bslot_l.append(bs)

    cnt = np.bincount(batch, minlength=NG).astype(np.float32)
    invcnt = 1.0 / np.clip(cnt, 1.0, None)
    return dict(l1=l1, l2=l2, bslot=bslot_l, invcnt=invcnt)


def fold_weights(inp):
    g = lambda k: np.asarray(inp[k], np.float32)
    W1, as1, ad1, b1 = g("W1"), g("att_src1"), g("att_dst1"), g("b1")
    W2, as2, ad2, b2 = g("W2"), g("att_src2"), g("att_dst2"), g("b2")
    g1, be1, rm1, rv1 = g("g1"), g("be1"), g("rm1"), g("rv1")
    g2, be2, rm2, rv2 = g("g2"), g("be2"), g("rm2"), g("rv2")
    lw1, lb1, lw2, lb2 = g("lw1"), g("lb1"), g("lw2"), g("lb2")

    def att_cols(W, a):
        return np.stack(
            [W[:, h * HID:(h + 1) * HID] @ a[h] for h in range(HEADS)], axis=1)

    Wcat1 = np.concatenate([W1, att_cols(W1, as1), att_cols(W1, ad1)], axis=1)
    s1 = g1 / np.sqrt(rv1 + EPS)
    t1 = be1 - rm1 * s1
    Wcat2u = np.concatenate([W2, att_cols(W2, as2), att_cols(W2, ad2)], axis=1)
    Wcat2 = s1[:, None] * Wcat2u
    rcat2 = t1 @ Wcat2u
    s2 = g2 / np.sqrt(rv2 + EPS)
    t2 = be2 - rm2 * s2
    return dict(
        wc1=_bf(Wcat1), wc2=_bf(Wcat2),
        rc2rep=_f32(np.tile(rcat2[None, :], (128, 1))),
        b1rep=_f32(np.tile(b1[None, :], (128, 1))),
        b2rep=_f32(np.tile(b2[None, :], (128, 1))),
        lw1=_bf(s2[:, None] * lw1), lb1=_f32((t2 @ lw1 + lb1)[:, None]),
        lw2=_bf(lw2), lb2=_f32(lb2[:, None]),
    )


# ------------------------------------------------------------- bass program
def build_program(l1m, l2m, GMAX):
    nc = bacc.Bacc("TRN2", num_devices=NCORES)

    ein = lambda name, shape, dt: nc.dram_tensor(name, shape, dt, kind="ExternalInput")
    xbT = ein("xbT", [128, NPAD], BF16)
    xbTo = ein("xbTo", [128, NPAD_LOC], BF16)
    wc1 = ein("wc1", [128, UCOLS], BF16)
    wc2 = ein("wc2", [256, UCOLS], BF16)
    rc2rep = ein("rc2rep", [128, UCOLS], F32)
    b1rep = ein("b1rep", [128, HC], F32)
    b2rep = ein("b2rep", [128, HC], F32)
    lw1 = ein("lw1", [256, OUT_DIM], BF16)
    lb1 = ein("lb1", [OUT_DIM, 1], F32)
    lw2 = ein("lw2", [OUT_DIM, N_CLS], BF16)
    lb2 = ein("lb2", [N_CLS, 1], F32)
    icntrep = ein("icntrep", [128, NG], F32)
    irepb = ein("irepb", [128, GMAX * 128], BF16)
    identb = ein("identb", [128, 128], BF16)
    i5rep = ein("i5rep", [128, NG], F32)
    idx16_1 = ein("idx16_1", [128, l1m["L"] // 16], I16)
    idx16_2 = ein("idx16_2", [128, l2m["L"] // 16], I16)
    ds1 = ein("ds1", [128, l1m["CH"]], BF16)
    ds2 = ein("ds2", [128, l2m["CH"]], BF16)
    oTt1 = ein("oTt1", [128, l1m["L"]], BF16)
    oTt2 = ein("oTt2", [128, l2m["L"]], BF16)
    bslot = ein("bslot", [128, NBLK], F32)
    out_t = nc.dram_tensor("out", [NG, N_CLS], F32, kind="ExternalOutput")

    T1 = nc.dram_tensor("T1", [NPAD, TCOLS], BF16)
    T2lA = nc.dram_tensor("T2lA", [ROWS_A, TCOLS], BF16)
    T2lB = nc.dram_tensor("T2lB", [ROWS_B, TCOLS], BF16)
    T2A = nc.dram_tensor("T2A", [NCORES * ROWS_A, TCOLS], BF16, addr_space="Shared")
    T2BC = nc.dram_tensor("T2BC", [NCORES * ROWS_B, TCOLS], BF16, addr_space="Shared")
    plcl = nc.dram_tensor("plcl", [2, HC, NG], F32)
    prdc = nc.dram_tensor("prdc", [2, HC, NG], F32, addr_space="Shared")

    from contextlib import ExitStack
    with tile.TileContext(nc) as tc, ExitStack() as es:
        cp = es.enter_context(tc.tile_pool(name="cp", bufs=1))
        wp = es.enter_context(tc.tile_pool(name="wp", bufs=3))
        gp = es.enter_context(tc.tile_pool(name="gp", bufs=4))
        pp = es.enter_context(tc.tile_pool(name="pp", bufs=1, space="PSUM"))
        pp2 = es.enter_context(tc.tile_pool(name="pp2", bufs=2, space="PSUM"))

        # ---- constants into SBUF
        def cload(ap, shape, dt, tag):
            t = cp.tile(shape, dt, tag=tag)
            nc.sync.dma_start(out=t[:], in_=ap)
            return t

        wc1_s = cload(wc1[:, :], [128, UCOLS], BF16, "wc1")
        wc2_s = cload(wc2[:, :].rearrange("(k p) c -> p k c", p=128), [128, 2, UCOLS], BF16, "wc2")
        rc2_s = cload(rc2rep[:, :], [128, UCOLS], F32, "rc2")
        b1_s = cload(b1rep[:, :], [128, HC], F32, "b1")
        b2_s = cload(b2rep[:, :], [128, HC], F32, "b2")
        lw1_s = cload(lw1[:, :].rearrange("(k p) c -> p k c", p=128), [128, 2, OUT_DIM], BF16, "lw1")
        lb1_s = cload(lb1[:, :], [OUT_DIM, 1], F32, "lb1")
        lw2_s = cload(lw2[:, :], [OUT_DIM, N_CLS], BF16, "lw2")
        lb2_s = cload(lb2[:, :], [N_CLS, 1], F32, "lb2")
        icnt_s = cload(icntrep[:, :], [128, NG], F32, "icnt")
        irepb_s = cload(irepb[:, :], [128, GMAX * 128], BF16, "irepb")
        ident_s = cload(identb[:, :], [128, 128], BF16, "ident")
        i5_s = cload(i5rep[:, :], [128, NG], F32, "i5")
        ds1_s = cload(ds1[:, :], [128, l1m["CH"]], BF16, "ds1")
        ds2_s = cload(ds2[:, :], [128, l2m["CH"]], BF16, "ds2")
        bs_s = cload(bslot[:, :], [128, NBLK], F32, "bs")

        # persistent: own T2 rows (written during L1, read in L2)
        T2own = cp.tile([128, NBLK, UCOLS], BF16, tag="T2own")
        adsb1 = cp.tile([128, NBLK, HEADS], BF16, tag="adsb1")

        # ---- phase B: T1 = xb @ Wcat1 (all nodes, replicated)
        for j in range(NPAD // 512):
            eng = nc.sync if j % 2 == 0 else nc.scalar
            xt = wp.tile([128, 512], BF16, tag="xt")
            eng.dma_start(out=xt[:], in_=xbT[:, j * 512:(j + 1) * 512])
            # full 768B rows (cols 264:384 unused garbage) -> the DRAM write
            # is fully contiguous, ~2x the effective write bandwidth
            tb4 = wp.tile([128, 4, TCOLS], BF16, tag="tb4")
            nc.gpsimd.memset(tb4[:, :, UCOLS:TCOLS], 0.0)
            for s in range(4):
                ps = pp2.tile([128, UCOLS], F32, tag="tb", space="PSUM")
                nc.tensor.matmul(out=ps[:], lhsT=xt[:, s * 128:(s + 1) * 128],
                                 rhs=wc1_s[:], start=True, stop=True)
                nc.vector.tensor_copy(out=tb4[:, s, 0:UCOLS], in_=ps[:])
            eng2 = nc.scalar if j % 2 == 0 else nc.sync
            # partition-major store: phys row p*392 + j*4 + s -> each
            # partition writes one contiguous 3KB run per iteration
            eng2.dma_start(
                out=T1[:, :].rearrange("(p r) c -> p r c", p=128)[
                    :, j * 4:(j + 1) * 4, :],
                in_=tb4[:])

        # ---- phase C: a_d(layer1) for own nodes
        for b in range(NBLK):
            xo = wp.tile([128, 128], BF16, tag="xo")
            nc.sync.dma_start(out=xo[:], in_=xbTo[:, b * 128:(b + 1) * 128])
            ps = pp2.tile([128, HEADS], F32, tag="adp", space="PSUM")
            nc.tensor.matmul(out=ps[:], lhsT=xo[:],
                             rhs=wc1_s[:, 260:264], start=True, stop=True)
            nc.scalar.activation(out=adsb1[:, b, :], in_=ps[:], func=AF.Copy)

        # ---- message-passing layer emitter
        def emit_layer(lm, idx_t, ds_s, oTt, tabA, tabB, adsb_of, brep_s,
                       post, self_fn, after_pair):
            for pi, info in enumerate(lm["pairs"]):
                q0, G, gA, gB = info["q0"], info["G"], info["gA"], info["gB"]
                e0 = q0 * 128
                idxp = gp.tile([128, G * 8], I16, tag="idxp")
                nc.sync.dma_start(out=idxp[:], in_=idx_t[:, e0 // 16:(e0 + G * 128) // 16])
                gbuf = gp.tile([128, G, TCOLS], BF16, tag="gbuf")
                if gA:
                    nc.gpsimd.dma_gather(
                        out_ap=gbuf[:, 0:gA, :], in_ap=tabA,
                        idxs_ap=idxp[:, 0:gA * 8],
                        num_idxs=gA * 128, num_idxs_reg=info["nidxA"],
                        elem_size=TCOLS, elem_step=TCOLS, single_packet=False)
                if gB:
                    nc.gpsimd.dma_gather(
                        out_ap=gbuf[:, gA:G, :], in_ap=tabB,
                        idxs_ap=idxp[:, gA * 8:G * 8],
                        num_idxs=gB * 128, num_idxs_reg=info["nidxB"],
                        elem_size=TCOLS, elem_step=TCOLS, single_packet=False)
                after_pair(pi)
                oT = gp.tile([128, G, 128], BF16, tag="oT")
                nc.sync.dma_start(out=oT[:], in_=oTt[:, q0 * 128:(q0 + G) * 128])
                og = gp.tile([128, G, 128], BF16, tag="og")
                nc.vector.tensor_tensor(
                    out=og[:], in0=ds_s[:, q0:q0 + G].broadcast_to([128, G, 128]),
                    in1=irepb_s[:, 0:G * 128].rearrange("p (g j) -> p g j", j=128),
                    op=OP.is_equal)

                # a_d expand per chunk: [128e, 4] = oT_g^T @ a_d_block
                adp = pp2.tile([128, G, HEADS], F32, tag="adp", space="PSUM")
                for g in range(G):
                    nc.tensor.matmul(out=adp[:, g, :], lhsT=oT[:, g, :],
                                     rhs=adsb_of(lm["chunk_block"][q0 + g]),
                                     start=True, stop=True)
                # scores -> w = exp(leaky(a_s + a_d))
                esb = wp.tile([128, G, HEADS], F32, tag="esb")
                nc.vector.tensor_tensor(out=esb[:], in0=adp[:],
                                        in1=gbuf[:, :, 256:260], op=OP.add)
                wsb = wp.tile([128, G, HEADS], F32, tag="wsb")
                nc.vector.scalar_tensor_tensor(out=wsb[:], in0=esb[:], scalar=SLOPE,
                                               in1=esb[:], op0=OP.mult, op1=OP.max)
                wex = wp.tile([128, G, HEADS], F32, tag="wex")
                nc.scalar.activation(out=wex[:], in_=wsb[:], func=AF.Exp)
                # w into table cols 256:260 (denominator columns), scale h by w
                nc.vector.tensor_copy(out=gbuf[:, :, 256:260], in_=wex[:])
                hview = gbuf[:, :, 0:256].rearrange("p g (h c) -> p g h c", h=HEADS)
                nc.vector.tensor_tensor(out=hview, in0=hview,
                                        in1=wex[:].broadcast_to([128, G, HEADS, HID]),
                                        op=OP.mult)
                # per block: aggregation + normalize + output
                for blk in info["blocks"]:
                    b = blk["b"]
                    chunks = [blk["a0"] - q0 + k for k in range(blk["na"])] + \
                             [blk["b0"] - q0 + k for k in range(blk["nb"])]
                    n_mm = len(chunks) + (1 if self_fn is not None else 0)
                    agg = pp2.tile([128, 260], F32, tag="agg", space="PSUM")
                    i = 0
                    if self_fn is not None:
                        selfc = self_fn(b)
                        nc.tensor.matmul(out=agg[:], lhsT=ident_s[:], rhs=selfc,
                                         start=True, stop=(n_mm == 1))
                        i = 1
                    for g in chunks:
                        nc.tensor.matmul(out=agg[:], lhsT=og[:, g, :],
                                         rhs=gbuf[:, g, 0:260],
                                         start=(i == 0), stop=(i == n_mm - 1))
                        i += 1
                    # out = num/den + b, relu -> bf16
                    den = wp.tile([128, HEADS], F32, tag="den")
                    nc.vector.tensor_scalar_add(out=den[:], in0=agg[:, 256:260],
                                                scalar1=1e-30)
                    rec = wp.tile([128, HEADS], F32, tag="rec")
                    nc.vector.reciprocal(out=rec[:], in_=den[:])
                    osb = wp.tile([128, HC], F32, tag="osb")
                    nc.vector.tensor_tensor(
                        out=osb[:].rearrange("p (h c) -> p h c", h=HEADS),
                        in0=agg[:, 0:256].rearrange("p (h c) -> p h c", h=HEADS),
                        in1=rec[:].broadcast_to([128, HEADS, HID]),
                        op=OP.mult)
                    nc.vector.tensor_tensor(out=osb[:], in0=osb[:], in1=brep_s[:],
                                            op=OP.add)
                    h2 = wp.tile([128, HC], BF16, tag="h2")
                    nc.scalar.activation(out=h2[:], in_=osb[:], func=AF.Relu)
                    post(b, h2)

        # ---- layer 1 (T2 row build + chunked AllGather folded in)
        def post1(b, h2):
            pt = pp2.tile([128, 2, 128], F32, tag="tb", space="PSUM")
            nc.tensor.matmul(out=pt[:, 0, :], lhsT=h2[:, 0:128], rhs=ident_s[:],
                             start=True, stop=True)
            nc.tensor.matmul(out=pt[:, 1, :], lhsT=h2[:, 128:256], rhs=ident_s[:],
                             start=True, stop=True)
            h2T = wp.tile([128, 2, 128], BF16, tag="h2T")
            nc.vector.tensor_copy(out=h2T[:], in_=pt[:])
            ps = pp2.tile([128, UCOLS], F32, tag="tb", space="PSUM")
            nc.tensor.matmul(out=ps[:], lhsT=h2T[:, 0, :], rhs=wc2_s[:, 0, :],
                             start=True, stop=False)
            nc.tensor.matmul(out=ps[:], lhsT=h2T[:, 1, :], rhs=wc2_s[:, 1, :],
                             start=False, stop=True)
            nc.vector.tensor_tensor(out=T2own[:, b, :], in0=ps[:], in1=rc2_s[:],
                                    op=OP.add)
            eng = nc.sync if b % 2 == 0 else nc.scalar
            if b < NBLK_A:
                eng.dma_start(out=T2lA[b * 128:(b + 1) * 128, 0:UCOLS],
                              in_=T2own[:, b, :])
            else:
                r0 = (b - NBLK_A) * 128
                rows = min(128, ROWS_B - r0)
                eng.dma_start(out=T2lB[r0:r0 + rows, 0:UCOLS],
                              in_=T2own[0:rows, b, :])

        grp = [list(range(NCORES))]

        def ag_chunk(k):
            if k == 0:
                nc.gpsimd.collective_compute(
                    "AllGather", OP.bypass, replica_groups=grp,
                    ins=[T2lA[:, :]], outs=[T2A[:, :]])
            else:
                off, n, base = BSEC_OFF[k - 1], BSEC[k - 1], BSEC_BASE[k - 1]
                nc.gpsimd.collective_compute(
                    "AllGather", OP.bypass, replica_groups=grp,
                    ins=[T2lB[off:off + n, :]],
                    outs=[T2BC[base:base + NCORES * n, :]])

        def after_pair1(pi):
            if pi in AG_AFTER_PAIR:
                ag_chunk(AG_AFTER_PAIR[pi])

        emit_layer(l1m, idx16_1, ds1_s, oTt1,
                   T1[0:SPLIT1, 0:TCOLS], T1[SPLIT1:2 * SPLIT1, 0:TCOLS],
                   lambda b: adsb1[:, b, :], b1_s, post1, None, after_pair1)
        ag_chunk(3)

        # ---- layer 2 + pooling (accumulated in two halves)
        plA = pp.tile([128, NG], F32, tag="plA", space="PSUM")
        plB = pp.tile([128, NG], F32, tag="plB", space="PSUM")
        plsb = cp.tile([128, 2, NG], F32, tag="plsb")

        def flush_pool(half):
            nc.vector.tensor_copy(out=plsb[:, 0, :], in_=plA[:])
            nc.vector.tensor_copy(out=plsb[:, 1, :], in_=plB[:])
            nc.sync.dma_start(
                out=plcl[half, :, :].rearrange("(k p) g -> p k g", p=128),
                in_=plsb[:, :, :])

        def post2(b, h2):
            Bm = wp.tile([128, NG], BF16, tag="Bm")
            nc.vector.tensor_tensor(
                out=Bm[:], in0=bs_s[:, b:b + 1].broadcast_to([128, NG]),
                in1=i5_s[:], op=OP.is_equal)
            seg_start = b in (0, POOL_SPLIT_BLK)
            seg_stop = b in (POOL_SPLIT_BLK - 1, NBLK - 1)
            nc.tensor.matmul(out=plA[:], lhsT=h2[:, 0:128], rhs=Bm[:],
                             start=seg_start, stop=seg_stop)
            nc.tensor.matmul(out=plB[:], lhsT=h2[:, 128:256], rhs=Bm[:],
                             start=seg_start, stop=seg_stop)
            if b == POOL_SPLIT_BLK - 1:
                flush_pool(0)

        def self2(b):
            esbS = wp.tile([128, HEADS], F32, tag="esbS")
            nc.vector.tensor_tensor(out=esbS[:], in0=T2own[:, b, 256:260],
                                    in1=T2own[:, b, 260:264], op=OP.add)
            wsbS = wp.tile([128, HEADS], F32, tag="wsbS")
            nc.vector.scalar_tensor_tensor(out=wsbS[:], in0=esbS[:], scalar=SLOPE,
                                           in1=esbS[:], op0=OP.mult, op1=OP.max)
            wexS = wp.tile([128, HEADS], F32, tag="wexS")
            nc.scalar.activation(out=wexS[:], in_=wsbS[:], func=AF.Exp)
            selfc = wp.tile([128, 260], BF16, tag="selfc")
            nc.vector.tensor_copy(out=selfc[:, 256:260], in_=wexS[:])
            nc.vector.tensor_tensor(
                out=selfc[:, 0:256].rearrange("p (h c) -> p h c", h=HEADS),
                in0=T2own[:, b, 0:256].rearrange("p (h c) -> p h c", h=HEADS),
                in1=wexS[:].broadcast_to([128, HEADS, HID]), op=OP.mult)
            return selfc[:, 0:260]

        def after_pair2(pi):
            if pi == AR1_AFTER_PAIR:
                nc.gpsimd.collective_compute(
                    "AllReduce", OP.add, replica_groups=[list(range(NCORES))],
                    ins=[plcl[0, :, :]], outs=[prdc[0, :, :]])

        emit_layer(l2m, idx16_2, ds2_s, oTt2,
                   T2A[:, :], T2BC[:, :],
                   lambda b: T2own[:, b, 260:264], b2_s, post2, self2,
                   after_pair2)
        flush_pool(1)
        nc.gpsimd.collective_compute(
            "AllReduce", OP.add, replica_groups=[list(range(NCORES))],
            ins=[plcl[1, :, :]], outs=[prdc[1, :, :]])

        # ---- pooled sum + MLP head
        prsb = cp.tile([128, 2, NG], F32, tag="prsb")
        prtot = cp.tile([128, 2, NG], F32, tag="prtot")
        nc.sync.dma_start(out=prtot[:],
                          in_=prdc[0, :, :].rearrange("(k p) g -> p k g", p=128))
        nc.sync.dma_start(out=prsb[:],
                          in_=prdc[1, :, :].rearrange("(k p) g -> p k g", p=128))
        nc.vector.tensor_tensor(out=prtot[:], in0=prtot[:],
                                in1=prsb[:], op=OP.add)
        pbn = cp.tile([128, 2, NG], BF16, tag="pbn")
        nc.vector.tensor_tensor(out=pbn[:, 0, :], in0=prtot[:, 0, :], in1=icnt_s[:], op=OP.mult)
        nc.vector.tensor_tensor(out=pbn[:, 1, :], in0=prtot[:, 1, :], in1=icnt_s[:], op=OP.mult)
        zp = pp2.tile([128, NG], F32, tag="adp", space="PSUM")
        nc.tensor.matmul(out=zp[:], lhsT=lw1_s[:, 0, :], rhs=pbn[:, 0, :], start=True, stop=False)
        nc.tensor.matmul(out=zp[:], lhsT=lw1_s[:, 1, :], rhs=pbn[:, 1, :], start=False, stop=True)
        zT = wp.tile([128, NG], BF16, tag="zT")
        nc.scalar.activation(out=zT[:], in_=zp[:], func=AF.Relu, bias=lb1_s[:])
        op_ = pp2.tile([N_CLS, NG], F32, tag="agg", space="PSUM")
        nc.tensor.matmul(out=op_[:], lhsT=lw2_s[:], rhs=zT[:], start=True, stop=True)
        ofin = wp.tile([N_CLS, NG], F32, tag="ofin")
        nc.scalar.activation(out=ofin[:], in_=op_[:], func=AF.Identity, bias=lb2_s[:])
        nc.sync.dma_start(out=out_t[:, :].rearrange("n c -> c n"), in_=ofin[:])

    nc.finalize()
    return nc


# ---------------------------------------------------------------- kernel()
def _prepare(inputs):
    inp = {k: np.asarray(v) for k, v in inputs.items()}
    prep = preprocess_graph(inp["edge_index"], inp["batch"])
    fw = fold_weights(inp)
    l1m, l2m = prep["l1"], prep["l2"]
    GMAX = max(max(i["G"] for i in l1m["pairs"]),
               max(i["G"] for i in l2m["pairs"]))

    nc = build_program(l1m, l2m, GMAX)

    x = np.asarray(inp["x"], np.float32)
    xbT_full = np.zeros((128, NPAD), bfloat16)
    xbT_full[:, :N] = x.T.astype(bfloat16)
    common = dict(
        xbT=xbT_full,
        wc1=fw["wc1"], wc2=fw["wc2"], rc2rep=fw["rc2rep"],
        b1rep=fw["b1rep"], b2rep=fw["b2rep"],
        lw1=fw["lw1"], lb1=fw["lb1"], lw2=fw["lw2"], lb2=fw["lb2"],
        icntrep=_f32(np.tile(prep["invcnt"][None, :], (128, 1))),
        irepb=_bf(np.tile(np.arange(128, dtype=np.float32)[None, :], (128, GMAX))),
        identb=_bf(np.eye(128, dtype=np.float32)),
        i5rep=_f32(np.tile(np.arange(NG, dtype=np.float32)[None, :], (128, 1))),
    )
    in_maps = []
    for c in range(NCORES):
        xo = np.zeros((128, NPAD_LOC), bfloat16)
        xo[:, :NLOC] = xbT_full[:, c * NLOC:(c + 1) * NLOC]
        in_maps.append(dict(
            common,
            xbTo=xo,
            idx16_1=np.ascontiguousarray(l1m["idx16"][c]),
            idx16_2=np.ascontiguousarray(l2m["idx16"][c]),
            ds1=np.ascontiguousarray(l1m["dslot"][c]),
            ds2=np.ascontiguousarray(l2m["dslot"][c]),
            oTt1=np.ascontiguousarray(l1m["oTt"][c]),
            oTt2=np.ascontiguousarray(l2m["oTt"][c]),
            bslot=np.ascontiguousarray(prep["bslot"][c]),
        ))
    return nc, in_maps


def kernel(**inputs):
    nc, in_maps = _prepare(inputs)
    res = bass_utils.run_bass_kernel_spmd(nc, in_maps, core_ids=list(range(NCORES)))
    return np.asarray(res.results[0]["out"], np.float32)


def profile_run(**inputs):
    """Run with NTFF profiling; returns (output, exec_time_ns)."""
    nc, in_maps = _prepare(inputs)
    res = bass_utils.run_bass_kernel_spmd(
        nc, in_maps, core_ids=list(range(NCORES)), trace=True)
    return np.asarray(res.results[0]["out"], np.float32), res.exec_time_ns


if __name__ == "__main__":
    rng = np.random.default_rng(0)
    ei = rng.integers(0, N, (2, 800000)).astype(np.int64)
    bt = np.sort(rng.integers(0, NG, N)).astype(np.int64)
    p = preprocess_graph(ei, bt)
    for k in ("l1", "l2"):
        m = p[k]
        print(k, "CH", m["CH"], "L", m["L"],
              "GMAX", max(i["G"] for i in m["pairs"]))
